# revision 1
# baseline (speedup 1.0000x reference)
"""HMM forward-algorithm loss on 8 NeuronCores (Bass/Tile).

Math: loss = -mean_n logsumexp_k(alpha_T[n,:]) for the HMM forward recursion
  alpha_t = logsumexp_k(alpha_{t-1} + tran) + emis[x[:,t-1]]
Computed in the linear domain as a product of per-step matrices
  p_T = diag(e_T) A diag(e_{T-1}) A ... diag(e_1) A p_0,   A = Texp^T
which is split EXACTLY at T/2: forward half from p_0, backward half acting on
ones from the left; loss_n = log(dot(f, b)) + accumulated log-scales.

Sharding: 8 cores = 4 row-groups x 2 directions (8 batch rows each, 2048
serial steps instead of 4096). The emission table (V=50000 x K=128,
column-normalized, x128-scaled) is built on every core from the raw inputs,
then rows are fetched with indirect-DMA gathers + PE transposes during the
scan. Periodic (R=16) renormalization keeps fp32 in range; all log-scale
corrections are summed on the host into the final scalar.
"""

import numpy as np

N, T, K, V = 32, 4096, 128, 50000
P = 128
NT = (V + P - 1) // P          # 391 vocab tiles, last partial
HALF = T // 2                  # 2048 serial steps per core
R = 16                         # renorm period
C0 = 40.0                      # fixed max-bound for the table softmax
NSLOT = HALF + 1               # e-slots per chain (incl. leading/trailing ones)
NROW = 8                       # batch rows per core

_CACHE = {}


def _build_nc():
    import concourse.bass as bass
    import concourse.mybir as mybir
    import concourse.tile as tile
    from concourse import bacc
    from concourse.masks import make_identity

    f32 = mybir.dt.float32
    bf16 = mybir.dt.bfloat16
    i32 = mybir.dt.int32
    AX = mybir.AxisListType.X
    EXP = mybir.ActivationFunctionType.Exp
    LN = mybir.ActivationFunctionType.Ln
    SUB = mybir.AluOpType.subtract

    nc = bacc.Bacc("TRN2", target_bir_lowering=False, debug=False, num_devices=8)

    tr_d = nc.dram_tensor("tr", [K, K], f32, kind="ExternalInput")
    emb_d = nc.dram_tensor("emb", [K, K], f32, kind="ExternalInput")
    voc_d = nc.dram_tensor("voc", [V, K], f32, kind="ExternalInput")
    sw_d = nc.dram_tensor("sw", [K, 1], f32, kind="ExternalInput")
    sb_d = nc.dram_tensor("sb", [K, 1], f32, kind="ExternalInput")
    dirf_d = nc.dram_tensor("dirf", [K, 1], f32, kind="ExternalInput")
    offs_d = nc.dram_tensor("offs", [NSLOT * NROW, 1], i32, kind="ExternalInput")

    outp_d = nc.dram_tensor("outp", [K, NROW], f32, kind="ExternalOutput")
    acc_d = nc.dram_tensor("acc", [1, NROW], f32, kind="ExternalOutput")
    statlog_d = nc.dram_tensor("statlog", [V, 1], f32, kind="ExternalOutput")

    table_d = nc.dram_tensor("table2", [V + 1, K], f32)  # internal; row V = ones

    with tile.TileContext(nc) as tc:
        with (
            tc.tile_pool(name="csb", bufs=1) as csb,
            tc.tile_pool(name="sb2", bufs=2) as sb2,
            tc.tile_pool(name="sb3", bufs=3) as sb3,
            tc.tile_pool(name="p_t", bufs=2, space="PSUM") as p_t,
            tc.tile_pool(name="p_m", bufs=2, space="PSUM") as p_m,
            tc.tile_pool(name="p_s", bufs=2, space="PSUM") as p_s,
        ):
            # ---------- constants ----------
            ident = csb.tile([P, P], dtype=f32)
            make_identity(nc, ident[:])
            ones_col = csb.tile([P, 1], dtype=f32)
            nc.vector.memset(ones_col[:], 1.0)
            ones_row = csb.tile([1, P], dtype=f32)
            nc.vector.memset(ones_row[:], 1.0)

            trt = csb.tile([P, P], dtype=f32)
            nc.sync.dma_start(out=trt[:], in_=tr_d[:, :])
            embi = csb.tile([P, P], dtype=f32)
            nc.sync.dma_start(out=embi[:], in_=emb_d[:, :])
            swt = csb.tile([P, 1], dtype=f32)
            nc.sync.dma_start(out=swt[:], in_=sw_d[:, :])
            sbt = csb.tile([P, 1], dtype=f32)
            nc.sync.dma_start(out=sbt[:], in_=sb_d[:, :])
            dirt = csb.tile([P, 1], dtype=f32)
            nc.sync.dma_start(out=dirt[:], in_=dirf_d[:, :])

            # embT = emb^T
            tp = p_t.tile([P, P], dtype=f32, tag="pt")
            nc.tensor.transpose(out=tp[:], in_=embi[:], identity=ident[:])
            embT = csb.tile([P, P], dtype=f32)
            nc.scalar.copy(out=embT[:], in_=tp[:])

            # Texp = softmax(rows of tr)
            rm = csb.tile([P, 1], dtype=f32)
            nc.vector.tensor_reduce(out=rm[:], in_=trt[:], axis=AX, op=mybir.AluOpType.max)
            nrm = csb.tile([P, 1], dtype=f32)
            nc.vector.tensor_scalar_mul(out=nrm[:], in0=rm[:], scalar1=-1.0)
            rs = csb.tile([P, 1], dtype=f32)
            eLt = csb.tile([P, P], dtype=f32)
            nc.scalar.activation(out=eLt[:], in_=trt[:], func=EXP, bias=nrm[:, :1], accum_out=rs[:, :1])
            rrs = csb.tile([P, 1], dtype=f32)
            nc.vector.reciprocal(out=rrs[:], in_=rs[:])
            Texp = csb.tile([P, P], dtype=f32)
            nc.vector.tensor_scalar_mul(out=Texp[:], in0=eLt[:], scalar1=rrs[:, :1])
            tp2 = p_t.tile([P, P], dtype=f32, tag="pt")
            nc.tensor.transpose(out=tp2[:], in_=Texp[:], identity=ident[:])
            TexpT = csb.tile([P, P], dtype=f32)
            nc.scalar.copy(out=TexpT[:], in_=tp2[:])

            # W = dirf*Texp + (1-dirf)*TexpT   (lhsT for the scan matmul)
            dif = csb.tile([P, P], dtype=f32)
            nc.vector.tensor_tensor(out=dif[:], in0=Texp[:], in1=TexpT[:], op=SUB)
            dif2 = csb.tile([P, P], dtype=f32)
            nc.vector.tensor_scalar_mul(out=dif2[:], in0=dif[:], scalar1=dirt[:, :1])
            W = csb.tile([P, P], dtype=f32)
            nc.vector.tensor_add(out=W[:], in0=dif2[:], in1=TexpT[:])

            # q0 = dirf*exp(start_w+start_b) + (1-dirf)*1, broadcast to 8 cols
            p0s = csb.tile([P, 1], dtype=f32)
            nc.vector.tensor_add(out=p0s[:], in0=swt[:], in1=sbt[:])
            p0e = csb.tile([P, 1], dtype=f32)
            nc.scalar.activation(out=p0e[:], in_=p0s[:], func=EXP)
            t1 = csb.tile([P, 1], dtype=f32)
            nc.vector.tensor_scalar_add(out=t1[:], in0=p0e[:], scalar1=-1.0)
            t2 = csb.tile([P, 1], dtype=f32)
            nc.vector.tensor_mul(out=t2[:], in0=t1[:], in1=dirt[:])
            initc = csb.tile([P, 1], dtype=f32)
            nc.vector.tensor_scalar_add(out=initc[:], in0=t2[:], scalar1=1.0)
            ones8 = csb.tile([P, NROW], dtype=f32)
            nc.vector.memset(ones8[:], 1.0)
            q0 = csb.tile([P, NROW], dtype=f32)
            nc.vector.tensor_scalar_mul(out=q0[:], in0=ones8[:], scalar1=initc[:, :1])

            acc = csb.tile([1, NROW], dtype=f32)
            nc.vector.memset(acc[:], 0.0)

            negc0 = csb.tile([P, 1], dtype=f32)
            nc.vector.memset(negc0[:], -C0)

            # ---------- pass 1: column sums s[k] = sum_v exp(L[v,k] - C0) ----------
            s_acc = p_s.tile([1, P], dtype=f32, tag="s")
            for g in range(NT):
                u = min(P, V - g * P)
                vt = sb2.tile([P, P], dtype=f32, tag="vt")
                nc.sync.dma_start(out=vt[:u, :], in_=voc_d[g * P : g * P + u, :])
                tpv = p_t.tile([P, P], dtype=f32, tag="pt")
                nc.tensor.transpose(out=tpv[:, :u], in_=vt[:u, :], identity=ident[:u, :u])
                vT = sb2.tile([P, P], dtype=f32, tag="vT")
                nc.scalar.copy(out=vT[:, :u], in_=tpv[:, :u])
                pl = p_m.tile([P, P], dtype=f32, tag="m")
                nc.tensor.matmul(out=pl[:u, :], lhsT=vT[:, :u], rhs=embT[:], start=True, stop=True)
                ex = sb2.tile([P, P], dtype=f32, tag="ex")
                nc.scalar.activation(out=ex[:u, :], in_=pl[:u, :], func=EXP, bias=negc0[:u, :1])
                nc.tensor.matmul(
                    out=s_acc[:1, :],
                    lhsT=ones_col[:u, :1],
                    rhs=ex[:u, :],
                    start=(g == 0),
                    stop=(g == NT - 1),
                    skip_group_check=True,
                )

            # mlB[u,k] = 1/s[k]  (broadcast via rank-1 matmul)
            rS = csb.tile([1, P], dtype=f32)
            nc.vector.reciprocal(out=rS[:], in_=s_acc[:1, :])
            opp = p_t.tile([P, P], dtype=f32, tag="pt")
            nc.tensor.matmul(out=opp[:], lhsT=ones_row[:1, :], rhs=rS[:1, :], start=True, stop=True)
            mlB = csb.tile([P, P], dtype=f32)
            nc.scalar.copy(out=mlB[:], in_=opp[:])

            # ---------- pass 2: table2[v,:] = 128*exp(L-lse)/css[v]; statlog[v]=log css ----------
            for g in range(NT):
                u = min(P, V - g * P)
                vt = sb2.tile([P, P], dtype=f32, tag="vt")
                nc.sync.dma_start(out=vt[:u, :], in_=voc_d[g * P : g * P + u, :])
                tpv = p_t.tile([P, P], dtype=f32, tag="pt")
                nc.tensor.transpose(out=tpv[:, :u], in_=vt[:u, :], identity=ident[:u, :u])
                vT = sb2.tile([P, P], dtype=f32, tag="vT")
                nc.scalar.copy(out=vT[:, :u], in_=tpv[:, :u])
                pl = p_m.tile([P, P], dtype=f32, tag="m")
                nc.tensor.matmul(out=pl[:u, :], lhsT=vT[:, :u], rhs=embT[:], start=True, stop=True)
                ex = sb2.tile([P, P], dtype=f32, tag="ex")
                nc.scalar.activation(out=ex[:u, :], in_=pl[:u, :], func=EXP, bias=negc0[:u, :1])
                el = sb2.tile([P, P], dtype=f32, tag="el")
                nc.vector.tensor_mul(out=el[:u, :], in0=ex[:u, :], in1=mlB[:u, :])
                css = sb2.tile([P, 1], dtype=f32, tag="css")
                nc.vector.tensor_reduce(out=css[:u, :], in_=el[:u, :], axis=AX, op=mybir.AluOpType.add)
                rc = sb2.tile([P, 1], dtype=f32, tag="rc")
                nc.vector.reciprocal(out=rc[:u, :], in_=css[:u, :])
                rc128 = sb2.tile([P, 1], dtype=f32, tag="rc128")
                nc.vector.tensor_scalar_mul(out=rc128[:u, :], in0=rc[:u, :], scalar1=128.0)
                tt = sb2.tile([P, P], dtype=f32, tag="tt")
                nc.vector.tensor_scalar_mul(out=tt[:u, :], in0=el[:u, :], scalar1=rc128[:u, :1])
                sl = sb2.tile([P, 1], dtype=f32, tag="sl")
                nc.scalar.activation(out=sl[:u, :], in_=css[:u, :], func=LN)
                nc.sync.dma_start(out=table_d[g * P : g * P + u, :], in_=tt[:u, :])
                nc.sync.dma_start(out=statlog_d[g * P : g * P + u, :], in_=sl[:u, :1])
            nc.sync.dma_start(out=table_d[V : V + 1, :], in_=ones_row[:1, :])

            # ---------- scan: 2048 x (mul e, matmul W) + final mul ----------
            import concourse.bass as _b

            NTILE_G = NSLOT * NROW // P  # 128 full gather tiles
            rem = NSLOT * NROW - NTILE_G * P  # 8 leftover slots
            pP = None
            step = 0
            for t in range(NTILE_G + 1):
                rows = P if t < NTILE_G else rem
                idx = sb3.tile([P, 1], dtype=i32, tag="idx")
                nc.sync.dma_start(out=idx[:rows, :], in_=offs_d[t * P : t * P + rows, :])
                gt = sb3.tile([P, P], dtype=f32, tag="gt")
                nc.gpsimd.indirect_dma_start(
                    out=gt[:rows, :],
                    out_offset=None,
                    in_=table_d[:, :],
                    in_offset=_b.IndirectOffsetOnAxis(ap=idx[:rows, :1], axis=0),
                )
                tpg = p_t.tile([P, P], dtype=f32, tag="pt")
                nc.tensor.transpose(out=tpg[:, :rows], in_=gt[:rows, :], identity=ident[:rows, :rows])
                eT = sb3.tile([P, P], dtype=f32, tag="eT")
                nc.scalar.copy(out=eT[:, :rows], in_=tpg[:, :rows])

                nsteps = rows // NROW
                for j in range(nsteps):
                    ecols = eT[:, j * NROW : (j + 1) * NROW]
                    q = sb3.tile([P, NROW], dtype=f32, tag="q")
                    src = q0[:] if step == 0 else pP[:]
                    nc.vector.tensor_mul(out=q[:], in0=src, in1=ecols)
                    if step == HALF:
                        # final mul: q is the chain output
                        nc.sync.dma_start(out=outp_d[:, :], in_=q[:])
                        break
                    if step % R == R - 1:
                        cs = p_s.tile([1, NROW], dtype=f32, tag="s")
                        nc.tensor.matmul(out=cs[:], lhsT=ones_col[:, :1], rhs=q[:], start=True, stop=True)
                        lcs = sb3.tile([1, NROW], dtype=f32, tag="lcs")
                        nc.scalar.activation(out=lcs[:], in_=cs[:], func=LN)
                        nc.vector.tensor_add(out=acc[:], in0=acc[:], in1=lcs[:])
                        rcs = sb3.tile([1, NROW], dtype=f32, tag="rcs")
                        nc.vector.reciprocal(out=rcs[:], in_=cs[:])
                        bc = p_s.tile([P, NROW], dtype=f32, tag="s")
                        nc.tensor.matmul(out=bc[:], lhsT=ones_row[:1, :], rhs=rcs[:1, :], start=True, stop=True)
                        qn = sb3.tile([P, NROW], dtype=f32, tag="q")
                        nc.vector.tensor_mul(out=qn[:], in0=q[:], in1=bc[:])
                        q = qn
                    pP = p_m.tile([P, NROW], dtype=f32, tag="m")
                    nc.tensor.matmul(out=pP[:], lhsT=W[:], rhs=q[:], start=True, stop=True)
                    step += 1

            nc.sync.dma_start(out=acc_d[:1, :], in_=acc[:1, :])

    if not nc.is_finalized():
        nc.finalize()
    return nc


def _get_nc():
    if "nc" not in _CACHE:
        _CACHE["nc"] = _build_nc()
    return _CACHE["nc"]


def _make_offsets(x):
    """Per-core e-slot index sequences (int32 row indices into table2)."""
    per_core = []
    for c in range(8):
        g = c % 4
        rows = np.arange(g * NROW, (g + 1) * NROW)
        idx = np.empty((NSLOT, NROW), np.int32)
        if c < 4:  # forward half: slots [ONES, x[:,0..2046]], final mul = x[:,2047]
            idx[0, :] = V
            idx[1:, :] = x[rows, 0:HALF].T.astype(np.int32)
        else:  # backward half: slots x[:,4095..2048], final mul = ONES
            idx[0:HALF, :] = x[rows, T - 1 : HALF - 1 : -1].T.astype(np.int32)
            idx[HALF, :] = V
        per_core.append(idx.reshape(-1, 1))
    return per_core


def kernel(x, start_w, start_b, cluster_trans_w, emb_cluster_w, cluster_vocab_w):
    from concourse.bass_utils import run_bass_kernel_spmd

    x = np.asarray(x)
    nc = _get_nc()
    offs = _make_offsets(x)

    tr = np.ascontiguousarray(cluster_trans_w[:, 0].reshape(K, K).astype(np.float32))
    emb = np.ascontiguousarray(emb_cluster_w.astype(np.float32))
    voc = np.ascontiguousarray(cluster_vocab_w.astype(np.float32))
    sw = np.ascontiguousarray(start_w.astype(np.float32).reshape(K, 1))
    sb = np.ascontiguousarray(start_b.astype(np.float32).reshape(K, 1))

    in_maps = []
    for c in range(8):
        dirf = np.full((K, 1), 1.0 if c < 4 else 0.0, np.float32)
        in_maps.append(
            {"tr": tr, "emb": emb, "voc": voc, "sw": sw, "sb": sb,
             "dirf": dirf, "offs": offs[c]}
        )
    res = run_bass_kernel_spmd(nc, in_maps, list(range(8))).results

    statlog = res[0]["statlog"][:, 0].astype(np.float64)
    losses = np.empty(N, np.float64)
    for c in range(4):
        f = res[c]["outp"].astype(np.float64)          # (K, 8)
        b = res[c + 4]["outp"].astype(np.float64)      # (K, 8)
        af = res[c]["acc"][0].astype(np.float64)       # (8,)
        ab = res[c + 4]["acc"][0].astype(np.float64)
        rows = np.arange(c * NROW, (c + 1) * NROW)
        dots = (f * b).sum(axis=0)                     # (8,)
        corr = statlog[x[rows]].sum(axis=1) - T * np.log(128.0)
        losses[rows] = np.log(dots) + af + ab + corr
    return np.float32(-losses.mean())



# revision 2
# speedup vs baseline: 1.0445x; 1.0445x over previous
"""HMM forward-algorithm loss on 8 NeuronCores (Bass/Tile) — v3.

v3 vs v2 (trace-driven):
 - bc broadcast matmuls in bf16 (were fp32 two-pass: ~2.6us PE per tile).
 - renorm: snapshot at local step j==2, factor applied to the SAME tile's
   last e-slice (cols 120:128) -> no cross-tile rq dependency stalls.
 - compact per-core table: only the ~14K vocab rows this core's chains
   touch (unique of its 8 batch rows' x), padded to 16384 (= 8 clean
   [128,2048] chunks); row 16384 holds s/128 for the "ones" slots.
 - s[k] (softmax normalizer over the FULL vocab) from a per-core V/8
   shard sweep + AllReduce[1,128] (USE_COLLECTIVE), else full sweep.
 - build fused in groups of 4 tiles: [128,512] PSUM L-tile, one ACT exp,
   one 512-col s-matmul, one [512,128] table write.
"""

import numpy as np

N, T, K, V = 32, 4096, 128, 50000
P = 128
HALF = T // 2
R = 16
C0 = 40.0
NSLOT = HALF + 1
NROW = 8
NTILE_G = NSLOT * NROW // P      # 128 full gather tiles
REM = NSLOT * NROW - NTILE_G * P # 8
CH_COLS = 2048
UNI = 16384                      # compact table rows (padded)
UCHUNK = UNI * K // (P * CH_COLS)  # 8 compact chunks
SH_VIEW = 392                    # per-core shard view rows (x16 voc rows)
SH_PAD = 3136                    # 8*392 view rows = 50176 voc rows global

USE_COLLECTIVE = True

_CACHE = {}


def _build_nc():
    import concourse.bass as bass
    import concourse.mybir as mybir
    import concourse.tile as tile
    from concourse import bacc
    from concourse.masks import make_identity

    f32 = mybir.dt.float32
    bf16 = mybir.dt.bfloat16
    i32 = mybir.dt.int32
    EXP = mybir.ActivationFunctionType.Exp
    LN = mybir.ActivationFunctionType.Ln
    COPY = mybir.ActivationFunctionType.Copy
    SUB = mybir.AluOpType.subtract

    nc = bacc.Bacc("TRN2", target_bir_lowering=False, debug=False, num_devices=8)

    tr_d = nc.dram_tensor("tr", [K, K], f32, kind="ExternalInput")
    emb_d = nc.dram_tensor("emb", [K, K], f32, kind="ExternalInput")
    vocs_d = nc.dram_tensor("vocs", [UNI * K // CH_COLS, CH_COLS], f32, kind="ExternalInput")
    vocsh_d = nc.dram_tensor("vocsh", [SH_VIEW, CH_COLS], f32, kind="ExternalInput")
    sw_d = nc.dram_tensor("sw", [K, 1], f32, kind="ExternalInput")
    sb_d = nc.dram_tensor("sb", [K, 1], f32, kind="ExternalInput")
    dirf_d = nc.dram_tensor("dirf", [K, 1], f32, kind="ExternalInput")
    offs_d = nc.dram_tensor("offs", [P, NTILE_G + 1], i32, kind="ExternalInput")

    outp_d = nc.dram_tensor("outp", [K, NROW], f32, kind="ExternalOutput")
    accr_d = nc.dram_tensor("accr", [1, NROW], f32, kind="ExternalOutput")
    lnacc_d = nc.dram_tensor("lnacc", [1, P], f32, kind="ExternalOutput")

    table_d = nc.dram_tensor("table2", [UNI + 1, K], bf16)
    sP_d = nc.dram_tensor("sP", [1, K], f32)
    sG_d = nc.dram_tensor("sG", [1, K], f32)

    with tile.TileContext(nc) as tc:
        with (
            tc.tile_pool(name="csb", bufs=1) as csb,
            tc.tile_pool(name="sbA", bufs=3) as sbA,
            tc.tile_pool(name="sbB", bufs=2) as sbB,
            tc.tile_pool(name="sbS", bufs=3) as sbS,
            tc.tile_pool(name="p_t", bufs=2, space="PSUM") as p_t,
            tc.tile_pool(name="p_m", bufs=2, space="PSUM") as p_m,
            tc.tile_pool(name="p_cs", bufs=2, space="PSUM") as p_cs,
            tc.tile_pool(name="p_bc", bufs=2, space="PSUM") as p_bc,
        ):
            # ---------- constants ----------
            ident = csb.tile([P, P], dtype=f32)
            make_identity(nc, ident[:])
            identb = csb.tile([P, P], dtype=bf16)
            nc.vector.tensor_copy(out=identb[:], in_=ident[:])
            onesb_col = csb.tile([P, 1], dtype=bf16)
            nc.vector.memset(onesb_col[:], 1.0)
            ones128_row = csb.tile([1, P], dtype=bf16)
            nc.vector.memset(ones128_row[:], 128.0)
            negc0 = csb.tile([P, 1], dtype=f32)
            nc.vector.memset(negc0[:], -C0)

            trt = csb.tile([P, P], dtype=f32)
            nc.sync.dma_start(out=trt[:], in_=tr_d[:, :])
            embi = csb.tile([P, P], dtype=f32)
            nc.sync.dma_start(out=embi[:], in_=emb_d[:, :])
            swt = csb.tile([P, 1], dtype=f32)
            nc.sync.dma_start(out=swt[:], in_=sw_d[:, :])
            sbt = csb.tile([P, 1], dtype=f32)
            nc.sync.dma_start(out=sbt[:], in_=sb_d[:, :])
            dirt = csb.tile([P, 1], dtype=f32)
            nc.sync.dma_start(out=dirt[:], in_=dirf_d[:, :])
            offs_sb = csb.tile([P, NTILE_G + 1], dtype=i32)
            nc.sync.dma_start(out=offs_sb[:], in_=offs_d[:, :])

            tp0 = p_t.tile([P, P], dtype=f32, tag="pt")
            nc.tensor.transpose(out=tp0[:], in_=embi[:], identity=ident[:])
            embTb = csb.tile([P, P], dtype=bf16)
            nc.scalar.copy(out=embTb[:], in_=tp0[:])

            rm = csb.tile([P, 1], dtype=f32)
            nc.vector.tensor_reduce(
                out=rm[:], in_=trt[:], axis=mybir.AxisListType.X, op=mybir.AluOpType.max
            )
            nrm = csb.tile([P, 1], dtype=f32)
            nc.vector.tensor_scalar_mul(out=nrm[:], in0=rm[:], scalar1=-1.0)
            rs = csb.tile([P, 1], dtype=f32)
            eLt = csb.tile([P, P], dtype=f32)
            nc.scalar.activation(
                out=eLt[:], in_=trt[:], func=EXP, bias=nrm[:, :1], accum_out=rs[:, :1]
            )
            rrs = csb.tile([P, 1], dtype=f32)
            nc.vector.reciprocal(out=rrs[:], in_=rs[:])
            Texp = csb.tile([P, P], dtype=f32)
            nc.vector.tensor_scalar_mul(out=Texp[:], in0=eLt[:], scalar1=rrs[:, :1])
            tp1 = p_t.tile([P, P], dtype=f32, tag="pt")
            nc.tensor.transpose(out=tp1[:], in_=Texp[:], identity=ident[:])
            TexpT = csb.tile([P, P], dtype=f32)
            nc.scalar.copy(out=TexpT[:], in_=tp1[:])
            dif = csb.tile([P, P], dtype=f32)
            nc.vector.tensor_tensor(out=dif[:], in0=Texp[:], in1=TexpT[:], op=SUB)
            dif2 = csb.tile([P, P], dtype=f32)
            nc.vector.tensor_scalar_mul(out=dif2[:], in0=dif[:], scalar1=dirt[:, :1])
            Wf = csb.tile([P, P], dtype=f32)
            nc.vector.tensor_add(out=Wf[:], in0=dif2[:], in1=TexpT[:])
            Wb = csb.tile([P, P], dtype=bf16)
            nc.vector.tensor_copy(out=Wb[:], in_=Wf[:])

            p0s = csb.tile([P, 1], dtype=f32)
            nc.vector.tensor_add(out=p0s[:], in0=swt[:], in1=sbt[:])
            p0e = csb.tile([P, 1], dtype=f32)
            nc.scalar.activation(out=p0e[:], in_=p0s[:], func=EXP)
            t1 = csb.tile([P, 1], dtype=f32)
            nc.vector.tensor_scalar_add(out=t1[:], in0=p0e[:], scalar1=-1.0)
            t2 = csb.tile([P, 1], dtype=f32)
            nc.vector.tensor_mul(out=t2[:], in0=t1[:], in1=dirt[:])
            initc = csb.tile([P, 1], dtype=f32)
            nc.vector.tensor_scalar_add(out=initc[:], in0=t2[:], scalar1=1.0)
            ones8 = csb.tile([P, NROW], dtype=f32)
            nc.vector.memset(ones8[:], 1.0)
            q0 = csb.tile([P, NROW], dtype=f32)
            nc.vector.tensor_scalar_mul(out=q0[:], in0=ones8[:], scalar1=initc[:, :1])

            lnacc_sb = csb.tile([1, P], dtype=f32)
            nc.vector.memset(lnacc_sb[:], 0.0)
            accr_sb = csb.tile([1, NROW], dtype=f32)
            nc.vector.memset(accr_sb[:], 0.0)

            # ---------- sweep helper: 4-tile fused groups ----------
            def sweep_chunk(src_d, ch_row, rows, do_s, s_acc, s_first, s_last,
                            table_base):
                vt = sbA.tile([P, CH_COLS], dtype=f32, tag="vt")
                nc.sync.dma_start(
                    out=vt[:rows, :], in_=src_d[ch_row : ch_row + rows, :]
                )
                vb = sbB.tile([P, CH_COLS], dtype=bf16, tag="vb")
                nc.vector.tensor_copy(out=vb[:rows, :], in_=vt[:rows, :])
                for g in range(4):
                    pl4 = p_m.tile([P, 512], dtype=f32, tag="m")
                    for i in range(4):
                        j = g * 4 + i
                        tpv = p_t.tile([P, P], dtype=bf16, tag="pt")
                        nc.tensor.transpose(
                            out=tpv[:, :rows],
                            in_=vb[:rows, j * P : (j + 1) * P],
                            identity=identb[:rows, :rows],
                        )
                        vTb = sbA.tile([P, P], dtype=bf16, tag="vTb")
                        nc.vector.tensor_copy(out=vTb[:, :rows], in_=tpv[:, :rows])
                        nc.tensor.matmul(
                            out=pl4[:rows, i * P : (i + 1) * P],
                            lhsT=vTb[:, :rows], rhs=embTb[:],
                            start=True, stop=True, skip_group_check=True,
                        )
                    ex4 = sbB.tile([P, 512], dtype=bf16, tag="ex4")
                    nc.scalar.activation(
                        out=ex4[:rows, :], in_=pl4[:rows, :],
                        func=EXP, bias=negc0[:rows, :1],
                    )
                    if do_s:
                        nc.tensor.matmul(
                            out=s_acc[:1, :],
                            lhsT=onesb_col[:rows, :1],
                            rhs=ex4[:rows, :],
                            start=s_first and (g == 0),
                            stop=s_last and (g == 3),
                            skip_group_check=True,
                        )
                    else:
                        base = table_base + g * 4 * rows
                        for i in range(4):
                            nc.sync.dma_start(
                                out=table_d[base + i * rows : base + (i + 1) * rows, :],
                                in_=ex4[:rows, i * P : (i + 1) * P],
                            )

            # ---------- pass A1: shard sweep for s ----------
            s_acc = p_cs.tile([1, 512], dtype=f32, tag="cs")
            sh_rows = [P, P, P, SH_VIEW - 3 * P]  # 128,128,128,8
            for ci, rows in enumerate(sh_rows):
                sweep_chunk(vocsh_d, ci * P, rows, True, s_acc,
                            s_first=(ci == 0), s_last=(ci == len(sh_rows) - 1),
                            table_base=0)

            # ---------- pass A2: compact table build ----------
            for ch in range(UCHUNK):
                sweep_chunk(vocs_d, ch * P, P, False, None, False, False,
                            table_base=ch * CH_COLS)

            # ---------- s: reduce, allreduce, postprocess ----------
            s4s = csb.tile([1, 512], dtype=f32)
            nc.scalar.copy(out=s4s[:], in_=s_acc[:1, :])
            s01 = csb.tile([1, P], dtype=f32)
            nc.vector.tensor_add(out=s01[:], in0=s4s[:1, 0:P], in1=s4s[:1, P : 2 * P])
            s23 = csb.tile([1, P], dtype=f32)
            nc.vector.tensor_add(
                out=s23[:], in0=s4s[:1, 2 * P : 3 * P], in1=s4s[:1, 3 * P : 4 * P]
            )
            s_row = csb.tile([1, P], dtype=f32)
            nc.vector.tensor_add(out=s_row[:], in0=s01[:], in1=s23[:])
            if USE_COLLECTIVE:
                nc.sync.dma_start(out=sP_d[:, :], in_=s_row[:1, :])
                nc.gpsimd.collective_compute(
                    "AllReduce",
                    mybir.AluOpType.add,
                    replica_groups=[list(range(8))],
                    ins=[sP_d[:, :]],
                    outs=[sG_d[:, :]],
                )
                sg = csb.tile([1, P], dtype=f32)
                nc.sync.dma_start(out=sg[:], in_=sG_d[:, :])
            else:
                sg = s_row
            rowVb = csb.tile([1, P], dtype=bf16)
            nc.vector.tensor_scalar_mul(out=rowVb[:], in0=sg[:], scalar1=1.0 / 128.0)
            nc.sync.dma_start(out=table_d[UNI : UNI + 1, :], in_=rowVb[:1, :])
            sT = p_t.tile([P, 1], dtype=f32, tag="pt")
            nc.tensor.transpose(out=sT[:, :1], in_=sg[:1, :], identity=ident[:1, :1])
            rS = csb.tile([P, 1], dtype=f32)
            nc.vector.reciprocal(out=rS[:], in_=sT[:, :1])

            # ---------- scan ----------
            import concourse.bass as _b

            pP = None
            step = 0
            for t in range(NTILE_G + 1):
                rows = P if t < NTILE_G else REM
                gt = sbS.tile([P, P], dtype=bf16, tag="gt")
                nc.gpsimd.indirect_dma_start(
                    out=gt[:rows, :],
                    out_offset=None,
                    in_=table_d[:, :],
                    in_offset=_b.IndirectOffsetOnAxis(ap=offs_sb[:rows, t : t + 1], axis=0),
                )
                tpg = p_t.tile([P, P], dtype=bf16, tag="pt")
                nc.tensor.transpose(
                    out=tpg[:, :rows], in_=gt[:rows, :], identity=identb[:rows, :rows]
                )
                tmp = sbS.tile([P, P], dtype=bf16, tag="tmp")
                nc.scalar.activation(
                    out=tmp[:, :rows], in_=tpg[:, :rows], func=COPY, scale=rS[:, :1]
                )
                cs1 = p_cs.tile([1, P], dtype=f32, tag="cs")
                nc.tensor.matmul(
                    out=cs1[:1, :rows], lhsT=onesb_col[:, :1], rhs=tmp[:, :rows],
                    start=True, stop=True, skip_group_check=True,
                )
                lncs = sbS.tile([1, P], dtype=f32, tag="lncs")
                nc.scalar.activation(out=lncs[:1, :rows], in_=cs1[:1, :rows], func=LN)
                nc.vector.tensor_add(
                    out=lnacc_sb[:1, :rows], in0=lnacc_sb[:1, :rows], in1=lncs[:1, :rows]
                )
                rcsf = sbS.tile([1, P], dtype=f32, tag="rcsf")
                nc.vector.reciprocal_approx_fast(out=rcsf[:1, :rows], in_=cs1[:1, :rows])
                rcs = sbS.tile([1, P], dtype=bf16, tag="rcs")
                nc.vector.tensor_copy(out=rcs[:1, :rows], in_=rcsf[:1, :rows])
                # main bc + e for cols [0, rows) or [0, 120) on full tiles
                emain = rows - NROW if rows == P else rows
                bc = p_bc.tile([P, P], dtype=f32, tag="bc")
                nc.tensor.matmul(
                    out=bc[:, :emain], lhsT=ones128_row[:1, :], rhs=rcs[:1, :emain],
                    start=True, stop=True, skip_group_check=True,
                )
                eT = sbS.tile([P, P], dtype=bf16, tag="eT")
                nc.vector.tensor_mul(
                    out=eT[:, :emain], in0=tmp[:, :emain], in1=bc[:, :emain]
                )

                nsteps = rows // NROW
                for j in range(nsteps):
                    ecols = eT[:, j * NROW : (j + 1) * NROW]
                    src = q0[:] if step == 0 else pP[:]
                    if step == HALF:
                        qfin = csb.tile([P, NROW], dtype=f32)
                        nc.vector.tensor_mul(out=qfin[:], in0=src, in1=ecols)
                        nc.sync.dma_start(out=outp_d[:, :], in_=qfin[:])
                        break
                    qt = sbS.tile([P, NROW], dtype=bf16, tag="qt")
                    nc.vector.tensor_mul(out=qt[:], in0=src, in1=ecols)
                    if step % R == 2 and rows == P:
                        csq = p_cs.tile([1, NROW], dtype=f32, tag="cs")
                        nc.tensor.matmul(
                            out=csq[:], lhsT=onesb_col[:, :1], rhs=qt[:],
                            start=True, stop=True, skip_group_check=True,
                        )
                        lnq = sbS.tile([1, NROW], dtype=f32, tag="lnq")
                        nc.scalar.activation(out=lnq[:], in_=csq[:], func=LN)
                        nc.vector.tensor_add(out=accr_sb[:], in0=accr_sb[:], in1=lnq[:])
                        rq = sbS.tile([1, NROW], dtype=bf16, tag="rq")
                        with nc.allow_low_precision(reason="bf16 renorm; logged fp32"):
                            nc.vector.reciprocal(out=rq[:], in_=csq[:])
                        rch = sbS.tile([1, NROW], dtype=bf16, tag="rch")
                        nc.vector.tensor_mul(
                            out=rch[:], in0=rcs[:1, emain:rows], in1=rq[:]
                        )
                        nc.tensor.matmul(
                            out=bc[:, emain:rows], lhsT=ones128_row[:1, :],
                            rhs=rch[:1, :],
                            start=True, stop=True, skip_group_check=True,
                        )
                        nc.vector.tensor_mul(
                            out=eT[:, emain:rows], in0=tmp[:, emain:rows],
                            in1=bc[:, emain:rows],
                        )
                    pP = p_m.tile([P, NROW], dtype=f32, tag="m")
                    nc.tensor.matmul(
                        out=pP[:], lhsT=Wb[:], rhs=qt[:], start=True, stop=True
                    )
                    step += 1

            nc.sync.dma_start(out=accr_d[:1, :], in_=accr_sb[:1, :])
            nc.sync.dma_start(out=lnacc_d[:1, :], in_=lnacc_sb[:1, :])

    if not nc.is_finalized():
        nc.finalize()
    return nc


def _get_nc():
    if "nc" not in _CACHE:
        _CACHE["nc"] = _build_nc()
    return _CACHE["nc"]


def _sigma_c(i):
    """Compact-table row -> stored position (8x[128,2048] chunk order)."""
    i = np.asarray(i, np.int64)
    return (i // 2048) * 2048 + (i % 16) * 128 + (i % 2048) // 16


def _make_in_maps(x, start_w, start_b, cluster_trans_w, emb_cluster_w, cluster_vocab_w):
    x = np.asarray(x).astype(np.int64)
    voc = np.asarray(cluster_vocab_w).astype(np.float32)
    tr = np.ascontiguousarray(
        np.asarray(cluster_trans_w)[:, 0].reshape(K, K).astype(np.float32)
    )
    emb = np.ascontiguousarray(np.asarray(emb_cluster_w).astype(np.float32))
    sw = np.ascontiguousarray(np.asarray(start_w).astype(np.float32).reshape(K, 1))
    sb = np.ascontiguousarray(np.asarray(start_b).astype(np.float32).reshape(K, 1))

    vpad = np.zeros((SH_PAD * 16, K), np.float32)
    vpad[:V] = voc
    vsh_all = vpad.reshape(SH_PAD, CH_COLS)

    in_maps = []
    for c in range(8):
        g = c % 4
        rows = np.arange(g * NROW, (g + 1) * NROW)
        xc = x[rows, 0:HALF] if c < 4 else x[rows, HALF:T]  # this core's half
        uniq = np.unique(xc)
        nu = uniq.size
        assert nu <= UNI
        vocs = np.zeros((UNI, K), np.float32)
        vocs[:nu] = voc[uniq]
        # slot-major indices with sigma_c(compact position)
        lut = _sigma_c(np.searchsorted(uniq, xc))  # [8, HALF] positions
        idx = np.empty((NSLOT, NROW), np.int64)
        if c < 4:
            idx[0, :] = UNI
            idx[1:, :] = lut[:, 0:HALF].T
        else:
            idx[0:HALF, :] = lut[:, ::-1].T
            idx[HALF, :] = UNI
        flat = idx.reshape(-1)
        padded = np.zeros(P * (NTILE_G + 1), np.int64)
        padded[: flat.size] = flat
        offs = np.ascontiguousarray(padded.reshape(NTILE_G + 1, P).T.astype(np.int32))

        dirf = np.full((K, 1), 1.0 if c < 4 else 0.0, np.float32)
        in_maps.append(
            {
                "tr": tr, "emb": emb,
                "vocs": np.ascontiguousarray(vocs.reshape(UNI * K // CH_COLS, CH_COLS)),
                "vocsh": np.ascontiguousarray(vsh_all[c * SH_VIEW : (c + 1) * SH_VIEW]),
                "sw": sw, "sb": sb, "dirf": dirf, "offs": offs,
            }
        )
    return in_maps


def _combine(res):
    losses = np.empty(N, np.float64)
    for c in range(4):
        f = res[c]["outp"].astype(np.float64)
        b = res[c + 4]["outp"].astype(np.float64)
        af = res[c]["accr"][0].astype(np.float64)
        ab = res[c + 4]["accr"][0].astype(np.float64)
        ln = (res[c]["lnacc"][0] + res[c + 4]["lnacc"][0]).astype(np.float64)
        corr_r = ln.reshape(16, NROW).sum(axis=0)
        rows = np.arange(c * NROW, (c + 1) * NROW)
        dots = (f * b).sum(axis=0)
        losses[rows] = np.log(dots) + af + ab + corr_r - T * np.log(128.0)
    return np.float32(-losses.mean())


def kernel(x, start_w, start_b, cluster_trans_w, emb_cluster_w, cluster_vocab_w):
    from concourse.bass_utils import run_bass_kernel_spmd

    nc = _get_nc()
    in_maps = _make_in_maps(
        x, start_w, start_b, cluster_trans_w, emb_cluster_w, cluster_vocab_w
    )
    res = run_bass_kernel_spmd(nc, in_maps, list(range(8))).results
    return _combine(res)
